# revision 36
# baseline (speedup 1.0000x reference)
"""Trainium2 Bass kernel for nn_DeformableBlock (deformable conv v1 block).

Contract: kernel(**inputs) takes FULL unsharded inputs, returns FULL output.
Sharding: data-parallel over batch (B=8 -> 8 NeuronCores, 1 batch each),
weights replicated.

Per-core algorithm (one batch, Cin=128, Cout=256, H=W=64):
  1. offset conv (3x3, pad 1) as 9 shifted matmuls -> offset [18, 4096]
  2. PE-transpose offsets to pixel-major [128 jp, 32 jt, 18]; compute bilinear
     gather indices + tap weights with DVE ops (floor via int-cast trick;
     x-taps gathered as adjacent row-pairs with clamp/swap weight logic)
  3. indices through DRAM into the dma_gather "wrapped" layout via HWDGE
     xbar transposes (the wrap is a 128x16 int16 transpose) + group replicate
  4. build xe_pair [4224, 256] bf16 in DRAM: row r = [xe[r], xe[r+64]] where
     xe is the image with one zero row on top (extended-row trick folds the
     y-clamp into a single index re = clip(y0+1,0,64)). One dma_gather per
     (k, half) with elem=512 bf16 (1 KB) fetches ALL FOUR bilinear taps of a
     pixel in ONE descriptor -> G [128 jp, 16 jtl, 4*128] bf16
  5. tap weighting with per-partition scalars (tensor_scalar +
     scalar_tensor_tensor accumulate) in pixel-major; PE transposes the
     [jp, c] tile to channel-major PSUM; ACT evacuates into val_k
  6. main conv: out[o, j] = sum_k W_k[o,:] @ val_k  (bf16 matmuls, fp32 PSUM)
  7. bias + ReLU on ScalarE, DMA out [256, 4096] f32
"""
import os
import sys
import numpy as np

try:
    import concourse.bass as bass
except ImportError:  # pragma: no cover
    sys.path.insert(0, '/opt/trn_rl_repo')
    import concourse.bass as bass
import concourse.bacc as bacc

import concourse.mybir as mybir
import concourse.tile as tile
from concourse import library_config
from concourse.bass_utils import run_bass_kernel_spmd

F32 = mybir.dt.float32
F32R = mybir.dt.float32r
BF16 = mybir.dt.bfloat16
I32 = mybir.dt.int32
I16 = mybir.dt.int16
ALU = mybir.AluOpType
ACTF = mybir.ActivationFunctionType

B, CIN, COUT, H, W = 8, 128, 256, 64, 64
HW = H * W          # 4096
NJT = HW // 128     # 32 pixel-major tiles
NK = 9
KY = [(-1), (-1), (-1), 0, 0, 0, 1, 1, 1]
KX = [(-1), 0, 1, (-1), 0, 1, (-1), 0, 1]
NHALF = 2
JH = HW // NHALF    # 2048 pixels per half

_CACHE = {}


def _split_multiwaits(nc, max_waits=1, kinds=None):
    """walrus CoreV3 codegen rejects control instructions carrying more
    than one sem-wait; split the excess into a chain of same-engine
    drains placed directly before the offender."""
    if kinds is None:
        kinds = (mybir.InstDrain,)
    n_split = 0
    for fn in nc.m.functions:
        for bb in fn.blocks:
            insts = list(bb.instructions)
            new = []
            changed = False
            for inst in insts:
                si = inst.sync_info
                if (isinstance(inst, kinds) and si is not None
                        and len(si.on_wait) > max_waits):
                    waits = list(si.on_wait)
                    pre, rest = waits[:-max_waits], waits[-max_waits:]
                    for i in range(0, len(pre), max_waits):
                        chunk = pre[i:i + max_waits]
                        d = mybir.InstDrain(
                            name=f"{inst.name}-wsplit{i}",
                            engine=inst.engine,
                            ins=[], outs=[],
                            sync_info=mybir.SyncInfo(
                                on_wait=chunk, on_update=[]),
                        )
                        new.append(d)
                        n_split += 1
                    inst.sync_info = mybir.SyncInfo(
                        on_wait=rest, on_update=list(si.on_update))
                    changed = True
                new.append(inst)
            if changed:
                bb.instructions = new
    return n_split


def _build_program(phase=3):
    nc = bacc.Bacc('TRN2', target_bir_lowering=False, debug=False,
                   enable_asserts=False, num_devices=B,
                   num_swdge_queues=4)

    # ---- DRAM I/O ----
    xp_d = nc.dram_tensor('xp', [CIN, 66 * 66], F32, kind='ExternalInput')
    woffT_d = nc.dram_tensor('woffT', [9, CIN, 18], BF16, kind='ExternalInput')
    boff_d = nc.dram_tensor('boff', [18, 1], F32, kind='ExternalInput')
    wdefT_d = nc.dram_tensor('wdefT', [NK, CIN, COUT], BF16, kind='ExternalInput')
    bdef_d = nc.dram_tensor('bdef', [128, 2], F32, kind='ExternalInput')
    ident_d = nc.dram_tensor('ident', [128, 128], F32, kind='ExternalInput')
    hgk_d = nc.dram_tensor('hgk', [128, NJT, NK], F32, kind='ExternalInput')
    wgk_d = nc.dram_tensor('wgk', [128, NJT, NK], F32, kind='ExternalInput')
    y_d = nc.dram_tensor('y', [COUT, HW], F32, kind='ExternalOutput')

    # DRAM scratch: xe_pair row r = [xe_flat[r], xe_flat[r+64]] where
    # xe_flat = image extended with one zero row on top (66 rows total incl.
    # the zero tail).  4224 = 66 * 64.
    xep_d = nc.dram_tensor('xep_scratch', [4224, 2 * CIN], BF16,
                           kind='Internal')

    with tile.TileContext(nc) as tc:
        with (
            tc.tile_pool(name='const', bufs=1) as cpool,
            tc.tile_pool(name='ps_small', bufs=2, space='PSUM') as ps_small,
            tc.tile_pool(name='ps_conv', bufs=4, space='PSUM') as ps_conv,
        ):
            # persistent small tensors
            wdefT = cpool.tile([CIN, NK, COUT], BF16, tag='wdefT')
            nc.sync.dma_start(
                wdefT[:], wdefT_d.ap().rearrange('k c o -> c k o'))
            bdef = cpool.tile([128, 2], F32, tag='bdef')
            nc.sync.dma_start(bdef[:], bdef_d.ap())
            idxwr = cpool.tile([128, NHALF, NK, JH // 16], I16, tag='idxwr')
            w4 = cpool.tile([128, NJT, NK, 4], F32, tag='w4')
            identb = cpool.tile([128, 128], BF16, tag='identb')

            # ======== phase 1: offsets, indices, weights, xT ========
            with tc.tile_pool(name='p1', bufs=1) as apool:
                x_sb = apool.tile([CIN, 66 * 66], F32, tag='x_sb')
                nc.sync.dma_start(x_sb[:], xp_d.ap())
                xbf = apool.tile([CIN, 66 * 66], BF16, tag='xbf')
                nc.vector.tensor_copy(xbf[:], x_sb[:])
                ident = apool.tile([128, 128], F32, tag='ident')
                nc.sync.dma_start(ident[:], ident_d.ap())
                woffT = apool.tile([CIN, 9, 18], BF16, tag='woffT')
                nc.sync.dma_start(
                    woffT[:], woffT_d.ap().rearrange('s c o -> c s o'))
                boff = apool.tile([18, 1], F32, tag='boff')
                nc.sync.dma_start(boff[:], boff_d.ap())
                hgk = apool.tile([128, NJT, NK], F32, tag='hgk')
                nc.sync.dma_start(hgk[:], hgk_d.ap())
                wgk = apool.tile([128, NJT, NK], F32, tag='wgk')
                nc.sync.dma_start(wgk[:], wgk_d.ap())
                nc.vector.tensor_copy(identb[:], ident[:])

                # ---------- offset conv: off [18, 4096] f32 ----------
                # Conv runs on the padded 66-wide grid so the streaming
                # operand is a single contiguous run; the interior is
                # extracted in the ACT epilogue (multi-dim APs are fine
                # on ACT, just not on the PE streaming side).
                off_sb = apool.tile([18, HW], F32, tag='off_sb')
                chunks = [(1 + 7 * i, 7) for i in range(9)] + [(64, 1)]
                for r0, nr in chunks:
                    nfree = 66 * (nr - 1) + 64
                    ps = ps_small.tile([18, 512], F32, tag='ps', name='ps')
                    for s in range(9):
                        dh, dw = s // 3, s % 3
                        beg = r0 * 66 + 1 + (dh - 1) * 66 + (dw - 1)
                        rhs = bass.AP(
                            tensor=xbf[:].tensor,
                            offset=xbf[:].offset + beg,
                            ap=[list(xbf[:].ap[0]), [1, nfree]],
                        )
                        nc.tensor.matmul(
                            ps[:, :nfree], lhsT=woffT[:, s, :], rhs=rhs,
                            start=(s == 0), stop=(s == 8))
                    src_in = bass.AP(
                        tensor=ps[:].tensor, offset=ps[:].offset,
                        ap=[list(ps[:].ap[0]), [66, nr], [1, 64]])
                    nc.scalar.activation(
                        off_sb[:, 64 * (r0 - 1):64 * (r0 - 1 + nr)], src_in,
                        ACTF.Identity, bias=boff[:], scale=1.0)

                # ---------- transpose offsets to pixel-major ----------
                offT = apool.tile([128, NJT, 18], F32, tag='offT')
                for jt in range(NJT):
                    ps = ps_small.tile([128, 18], F32, tag='ps')
                    nc.tensor.transpose(
                        ps[:], off_sb[:, 128 * jt:128 * (jt + 1)],
                        ident[:18, :18])
                    nc.vector.tensor_copy(offT[:, jt, :], ps[:])

                # ---------- xe_pair build (bf16, [4224, 256] in DRAM) ------
                # xcp = [zeros(64) | pixels 0..4095 | zeros(128)] bf16 so
                # that xe_pair row r = [xcp[r], xcp[64+r]].  66 stride-33
                # PE transposes put 33 consecutive rows in each partition
                # (xTj[p, i] = row 33p+i) -> ONE DMA write with 128
                # contiguous 16.9 KB descriptors instead of 8192 x 256 B.
                xcp = apool.tile([CIN, 4288], BF16, tag='xcp')
                nc.vector.memset(xcp[:, 0:64], 0)
                nc.vector.memset(xcp[:, 4160:4288], 0)
                xin_all = bass.AP(
                    tensor=xbf[:].tensor,
                    offset=xbf[:].offset + 67,
                    ap=[list(xbf[:].ap[0]), [66, 64], [1, 64]])
                nc.vector.tensor_copy(xcp[:, 64:4160], xin_all)
                xTj = apool.tile([128, 33, 2 * CIN], BF16, tag='xTj')
                for i in range(33):
                    for blk in range(2):
                        src = bass.AP(
                            tensor=xcp[:].tensor,
                            offset=xcp[:].offset + 64 * blk + i,
                            ap=[list(xcp[:].ap[0]), [33, 128]])
                        ps = ps_small.tile([128, 128], BF16, tag='psb',
                                           name='ps')
                        nc.tensor.transpose(ps[:], src, identb[:])
                        nc.scalar.activation(
                            xTj[:, i, blk * CIN:(blk + 1) * CIN], ps[:],
                            ACTF.Copy)
                nc.sync.dma_start(
                    xep_d.ap().rearrange('(p i) cc -> p i cc', i=33),
                    xTj[:])

                # ---------- index/weight arithmetic (pixel-major) ----------
                sh = [128, NJT, NK]

                def T(tag, dt=F32):
                    return apool.tile(sh, dt, tag=tag, name=tag)

                dyx = offT[:].rearrange('p jt (k two) -> p jt k two', two=2)
                dy = dyx[:, :, :, 0]
                dx = dyx[:, :, :, 1]

                ti = apool.tile(sh, I32, tag='ti')
                fdy, fdx = T('fdy'), T('fdx')
                tmp1, tmp2, tmp3 = T('tmp1'), T('tmp2'), T('tmp3')
                # floor(dy)
                nc.vector.tensor_copy(ti[:], dy)
                nc.vector.tensor_copy(fdy[:], ti[:])
                nc.vector.tensor_tensor(tmp1[:], fdy[:], dy, ALU.is_gt)
                nc.vector.tensor_tensor(fdy[:], fdy[:], tmp1[:], ALU.subtract)
                # floor(dx)
                nc.vector.tensor_copy(ti[:], dx)
                nc.vector.tensor_copy(fdx[:], ti[:])
                nc.vector.tensor_tensor(tmp1[:], fdx[:], dx, ALU.is_gt)
                nc.vector.tensor_tensor(fdx[:], fdx[:], tmp1[:], ALU.subtract)

                ly, lx = T('ly'), T('lx')
                nc.vector.tensor_tensor(ly[:], dy, fdy[:], ALU.subtract)
                nc.vector.tensor_tensor(lx[:], dx, fdx[:], ALU.subtract)

                y0, x0 = T('y0'), T('x0')
                nc.vector.tensor_tensor(y0[:], hgk[:], fdy[:], ALU.add)
                nc.vector.tensor_tensor(x0[:], wgk[:], fdx[:], ALU.add)

                yc0, yc1, y01 = T('yc0'), T('yc1'), T('y01')
                nc.vector.tensor_scalar(yc0[:], y0[:], 63.0, 0.0, ALU.min, ALU.max)
                nc.vector.tensor_scalar(y01[:], y0[:], 1.0, None, ALU.add)
                nc.vector.tensor_scalar(yc1[:], y01[:], 63.0, 0.0, ALU.min, ALU.max)
                vy0, vy1 = T('vy0'), T('vy1')
                nc.vector.tensor_tensor(vy0[:], y0[:], yc0[:], ALU.is_equal)
                nc.vector.tensor_tensor(vy1[:], y01[:], yc1[:], ALU.is_equal)
                # extended-row index: re = clip(y0+1, 0, 64); the gather pair
                # (xe[re], xe[re+1]) then covers rows (y0, y0+1) with the
                # correct clamp semantics (out-of-range taps land on zero pad
                # or a garbage row whose weight is exactly 0).
                re_ = T('re_')
                nc.vector.tensor_scalar(re_[:], y01[:], 64.0, 0.0,
                                        ALU.min, ALU.max)

                bx, x01, e0, e1, e3 = T('bx'), T('x01'), T('e0'), T('e1'), T('e3')
                nc.vector.tensor_scalar(bx[:], x0[:], 62.0, 0.0, ALU.min, ALU.max)
                nc.vector.tensor_scalar(x01[:], x0[:], 1.0, None, ALU.add)
                nc.vector.tensor_tensor(e0[:], x0[:], bx[:], ALU.is_equal)
                nc.vector.tensor_tensor(e1[:], x01[:], bx[:], ALU.is_equal)
                nc.vector.tensor_scalar(tmp1[:], bx[:], 1.0, None, ALU.add)
                nc.vector.tensor_tensor(e3[:], x0[:], tmp1[:], ALU.is_equal)

                wy0, wy1 = T('wy0'), T('wy1')
                nc.vector.tensor_tensor(tmp1[:], ly[:], vy0[:], ALU.mult)
                nc.vector.tensor_tensor(wy0[:], vy0[:], tmp1[:], ALU.subtract)
                nc.vector.tensor_tensor(wy1[:], ly[:], vy1[:], ALU.mult)

                wx0, wx1 = T('wx0'), T('wx1')
                nc.vector.tensor_tensor(tmp1[:], lx[:], e0[:], ALU.mult)
                nc.vector.tensor_tensor(tmp2[:], lx[:], e1[:], ALU.mult)
                nc.vector.tensor_tensor(wx0[:], e0[:], tmp1[:], ALU.subtract)
                nc.vector.tensor_tensor(wx0[:], wx0[:], tmp2[:], ALU.add)
                nc.vector.tensor_tensor(tmp2[:], lx[:], e3[:], ALU.mult)
                nc.vector.tensor_tensor(tmp3[:], e3[:], tmp2[:], ALU.subtract)
                nc.vector.tensor_tensor(wx1[:], tmp1[:], tmp3[:], ALU.add)

                # tap weights -> w4 [128, NJT, NK, 4] f32 (persistent).
                # Order matches the 1 KB gather element:
                # [tap(y0,x0), tap(y0+1,x0), tap(y0,x0+1), tap(y0+1,x0+1)]
                nc.vector.tensor_tensor(w4[:, :, :, 0], wy0[:], wx0[:], ALU.mult)
                nc.vector.tensor_tensor(w4[:, :, :, 1], wy1[:], wx0[:], ALU.mult)
                nc.vector.tensor_tensor(w4[:, :, :, 2], wy0[:], wx1[:], ALU.mult)
                nc.vector.tensor_tensor(w4[:, :, :, 3], wy1[:], wx1[:], ALU.mult)

                # gather index -> idp [128, NJT, NK] f32 (exact ints):
                # idx = re*64 + bx in [0, 4158]
                idp = T('idp')
                nc.vector.tensor_scalar(tmp1[:], re_[:], 64.0, None, ALU.mult)
                nc.vector.tensor_tensor(idp[:], tmp1[:], bx[:], ALU.add)

                # ---------- wrap idx via identity-slice matmuls ----------
                # wrapped idx: [128, NHALF, NK, JH/16] int16 with
                # idxwr[p, h, k, jtl*8+g] = idp[16g+p, h*16+jtl, k]
                # (token i of (h,k): partition i%16, free i//16).  Moving
                # partition slice [16g,16g+16) to partitions 0-15 is one
                # 16-contract PE matmul with an identity slice; DVE then
                # scatters (cast f32->i16) into the strided wrap position.
                # Gathers rotate SWDGE queues 0-3: queue q is serviced by
                # Q7 cores 2q (rx) / 2q+1 (tx), so all 8 groups get the
                # idx data.
                idxwr_ap = idxwr[:]
                for g in range(8):
                    b = (g // 4) * 64
                    ps = ps_small.tile([16, NJT * NK], F32, tag='ps',
                                       name='ps')
                    # lhsT = ident[b.., 16g..] selects partition 16g+p out
                    # of the 64-partition slice starting at base b (PE
                    # tile bases are restricted to 0/64 for 64-row tiles).
                    nc.tensor.matmul(
                        ps[:], lhsT=ident[b:b + 64, 16 * g:16 * g + 16],
                        rhs=idp[b:b + 64, :, :].rearrange(
                            'p a b -> p (a b)'),
                        start=True, stop=True)
                    dst = bass.AP(
                        tensor=idxwr_ap.tensor, offset=idxwr_ap.offset + g,
                        ap=[[idxwr_ap.ap[0][0], 16],
                            [NK * (JH // 16), NHALF],
                            [8, 16], [JH // 16, NK]])
                    nc.vector.tensor_copy(
                        dst, ps[:].rearrange('p (h jtl k) -> p h jtl k',
                                             h=NHALF, jtl=16))
                for g0 in range(16, 128, 16):
                    nc.sync.dma_start(idxwr[g0:g0 + 16], idxwr[0:16])

            if phase != 1:
                _phase2(nc, tc, ps_small, ps_conv, wdefT, bdef, idxwr, w4,
                        identb, xep_d, y_d, phase)
            if phase != 3:
                with tc.tile_pool(name='zz', bufs=1) as zp:
                    zt = zp.tile([128, HW], F32, tag='zt')
                    nc.vector.memset(zt[:], 0)
                    for oh in range(2):
                        nc.sync.dma_start(
                            y_d.ap()[128 * oh:128 * (oh + 1), :], zt[:])

    nc.finalize()
    _split_multiwaits(nc)
    return nc


def _phase2(nc, tc, ps_small, ps_conv, wdefT, bdef, idxwr, w4, identb,
            xep_d, y_d, phase=3):
            # ======== phase 2: gather + weighting + conv ========
            # Pixel-major 4-tap gather (1 KB descriptor per token covering
            # all four bilinear taps), per-partition scalar tap weighting
            # (scalar_tensor_tensor), PE transpose to channel-major, ACT
            # evacuates PSUM into val.
            with (
                tc.tile_pool(name='gath', bufs=4) as gpool,
                tc.tile_pool(name='tmp2p', bufs=3) as tpool,
                tc.tile_pool(name='val', bufs=10) as vpool,
                tc.tile_pool(name='outp', bufs=2) as opool,
            ):
                for half in range(NHALF):
                    j0 = half * JH
                    vals = []
                    for k in range(NK):
                        # G [128 jp, 16 jtl, 4*CIN] bf16; token i ->
                        # partition i%128 = jp, chunk i//128 = local jt.
                        # Free order: [tap00, tap10, tap01, tap11] * CIN
                        G = gpool.tile([128, JH // 128, 4 * CIN], BF16,
                                       tag='G', name='G')
                        in_ap = bass.AP(
                            tensor=xep_d, offset=0,
                            ap=[[2 * CIN, 4160], [1, 4 * CIN]],
                        )
                        nc.gpsimd.dma_gather(
                            out_ap=G[:],
                            in_ap=in_ap,
                            idxs_ap=idxwr[:, half, k, :],
                            num_idxs=JH,
                            num_idxs_reg=JH,
                            elem_size=4 * CIN,
                            elem_step=2 * CIN,
                            transpose=False,
                            single_packet=False,
                            queue_num=k % 4,
                        )
                        if phase == 21:
                            continue
                        val = vpool.tile([128, JH], BF16, tag='val')
                        for jtl in range(JH // 128):
                            jt = half * (JH // 128) + jtl
                            acc = tpool.tile([128, CIN], BF16, tag='acc',
                                             name='acc')
                            nc.vector.tensor_scalar(
                                acc[:], G[:, jtl, 0:CIN],
                                w4[:, jt, k, 0].unsqueeze(1), None, ALU.mult)
                            for t in (1, 2, 3):
                                nc.vector.scalar_tensor_tensor(
                                    acc[:], G[:, jtl, t * CIN:(t + 1) * CIN],
                                    w4[:, jt, k, t].unsqueeze(1), acc[:],
                                    ALU.mult, ALU.add)
                            ps = ps_small.tile([128, 128], BF16, tag='psb',
                                               name='ps')
                            nc.tensor.transpose(ps[:], acc[:], identb[:])
                            nc.scalar.activation(
                                val[:, 128 * jtl:128 * (jtl + 1)], ps[:],
                                ACTF.Copy)
                        vals.append(val)

                        if phase == 3:
                            # oh=0 conv accumulates incrementally per k so
                            # only the oh=1 matmuls remain after the last
                            # gather (shorter tail).
                            if k == 0:
                                psA = [ps_conv.tile([128, 512], F32,
                                                    tag='ps_conv',
                                                    name='ps_conv')
                                       for _ in range(JH // 512)]
                            for jc in range(JH // 512):
                                nc.tensor.matmul(
                                    psA[jc],
                                    lhsT=wdefT[:, k, 0:128],
                                    rhs=val[:, 512 * jc:512 * (jc + 1)],
                                    start=(k == 0), stop=(k == NK - 1))

                    if phase != 3:
                        continue
                    for jc in range(JH // 512):
                        yo = opool.tile([128, 512], F32, tag='yo')
                        nc.scalar.activation(
                            yo[:], psA[jc], ACTF.Relu,
                            bias=bdef[:, 0:1], scale=1.0)
                        nc.sync.dma_start(
                            y_d.ap()[0:128,
                                     j0 + 512 * jc:j0 + 512 * (jc + 1)],
                            yo[:])
                    for jc in range(JH // 512):
                        ps = ps_conv.tile([128, 512], F32, tag='ps_conv')
                        for k in range(NK):
                            nc.tensor.matmul(
                                ps[:],
                                lhsT=wdefT[:, k, 128:256],
                                rhs=vals[k][:, 512 * jc:512 * (jc + 1)],
                                start=(k == 0), stop=(k == NK - 1))
                        yo = opool.tile([128, 512], F32, tag='yo')
                        nc.scalar.activation(
                            yo[:], ps[:], ACTF.Relu,
                            bias=bdef[:, 1:2], scale=1.0)
                        nc.sync.dma_start(
                            y_d.ap()[128:256,
                                     j0 + 512 * jc:j0 + 512 * (jc + 1)],
                            yo[:])


def _host_prep(x, w_off, b_off, w_def, b_def):
    """Build per-core input maps."""
    x = np.asarray(x, np.float32)
    w_off = np.asarray(w_off, np.float32)
    b_off = np.asarray(b_off, np.float32)
    w_def = np.asarray(w_def, np.float32)
    b_def = np.asarray(b_def, np.float32)

    woffT = np.stack([w_off[:, :, s // 3, s % 3].T for s in range(9)])
    woffT = _to_bf16(np.ascontiguousarray(woffT, np.float32))  # [9, 128, 18]
    wdefT = np.stack([w_def[:, :, s // 3, s % 3].T for s in range(9)])
    wdefT = _to_bf16(np.ascontiguousarray(wdefT))             # [9, 128, 256]
    bdef2 = np.ascontiguousarray(b_def.reshape(2, 128).T)     # [128, 2]
    ident = np.eye(128, dtype=np.float32)

    jp = np.arange(128)[:, None, None]
    jt = np.arange(NJT)[None, :, None]
    kk = np.arange(NK)[None, None, :]
    j = jt * 128 + jp
    ky = np.array(KY, np.float32)[kk]
    kx = np.array(KX, np.float32)[kk]
    hgk = (j // 64).astype(np.float32) + ky
    wgk = (j % 64).astype(np.float32) + kx
    hgk = np.ascontiguousarray(np.broadcast_to(hgk, (128, NJT, NK)), np.float32)
    wgk = np.ascontiguousarray(np.broadcast_to(wgk, (128, NJT, NK)), np.float32)

    xp = np.pad(x, ((0, 0), (0, 0), (1, 1), (1, 1))).reshape(B, CIN, 66 * 66)

    shared = {
        'woffT': woffT,
        'boff': np.ascontiguousarray(b_off.reshape(18, 1)),
        'wdefT': wdefT,
        'bdef': bdef2,
        'ident': ident,
        'hgk': hgk,
        'wgk': wgk,
    }
    in_maps = []
    for b in range(B):
        m = dict(shared)
        m['xp'] = np.ascontiguousarray(xp[b])
        in_maps.append(m)
    return in_maps


def _to_bf16(a):
    import ml_dtypes
    return a.astype(ml_dtypes.bfloat16)


LAST_RESULTS = None


def _ensure_trace_support():
    """Register the NTFF profile hook that the slim agent image lacks, and
    stub out the artifact upload. Only used when KBENCH_TRACE is set."""
    import contextlib
    import ctypes
    import types

    import concourse.bass_utils as bu
    bu.upload_artifacts = lambda tmpdir: tmpdir

    if 'antenv.axon_hooks' in sys.modules:
        return
    so_path = '/opt/axon/libaxon_pjrt.so'
    if not os.path.exists(so_path):
        return
    lib = ctypes.CDLL(so_path)
    if not hasattr(lib, 'axon_start_nrt_profile'):
        return
    lib.axon_start_nrt_profile.argtypes = [
        ctypes.POINTER(ctypes.c_int64), ctypes.c_size_t]
    lib.axon_start_nrt_profile.restype = ctypes.c_int64
    lib.axon_stop_nrt_profile.argtypes = [ctypes.c_char_p]
    lib.axon_stop_nrt_profile.restype = ctypes.c_int64

    @contextlib.contextmanager
    def _hook(output_dir, device_ids):
        import jax
        jax.devices()
        if device_ids:
            ids = (ctypes.c_int64 * len(device_ids))(*device_ids)
            rc = lib.axon_start_nrt_profile(ids, len(device_ids))
        else:
            rc = lib.axon_start_nrt_profile(None, 0)
        if rc != 0:
            raise RuntimeError(f'axon_start_nrt_profile rc={rc}')
        try:
            yield
        finally:
            n = lib.axon_stop_nrt_profile(str(output_dir).encode())
            print(f'profile: {n} file(s) written to {output_dir}',
                  file=sys.stderr)

    mod = types.ModuleType('antenv.axon_hooks')
    mod.get_axon_ntff_profile_hook = lambda: _hook
    mod.set_axon_ntff_profile_hook = lambda h: None
    sys.modules['antenv.axon_hooks'] = mod


def kernel(x, w_off, b_off, w_def, b_def):
    global LAST_RESULTS
    if 'nc' not in _CACHE:
        _CACHE['nc'] = _build_program(
            phase=int(os.environ.get('KBENCH_PHASE', '3')))
    nc = _CACHE['nc']
    in_maps = _host_prep(x, w_off, b_off, w_def, b_def)
    trace = bool(os.environ.get('KBENCH_TRACE'))
    if trace:
        _ensure_trace_support()
    res = run_bass_kernel_spmd(
        nc, in_maps, core_ids=list(range(B)),
        trace=trace,
    )
    LAST_RESULTS = res
    out = np.stack([res.results[b]['y'].reshape(COUT, H, W) for b in range(B)])
    return out.astype(np.float32)

